# revision 57
# baseline (speedup 1.0000x reference)
"""Trainium2 Bass kernel for the char-CNN NLP model (data-parallel over 8 cores).

Pipeline:
  host:   emb = x @ emb_w (one-hot projection), laid out [cin, batch, seq],
          quantized to fp8e4 (scaled x64; TRN FP8_EXP4 == ml_dtypes.float8_e4m3)
  device: 3 parallel 1-D conv banks (k=2,3,4; 256 filters each) as fp8
          DoubleRow matmuls (two cin-chunks contracted per pass, fp32 PSUM);
          per (channel, batch) max over sequence; per channel sum of squares
          -> tiny stats tensor per core
  host:   batchnorm statistics from the factorized mean + device sumsq,
          monotone-affine BN+ReLU+maxpool reconstruction from max (min when
          some bn gamma < 0), fc1 -> bn -> relu -> fc2 -> softmax

BN(c+bias) is affine per channel, so max_t relu(bn(c)) = relu(s*M + t) with
M = max_t c if s>=0 else min_t c - exact, and the conv bias cancels inside BN.

Layout trick: each batch's sequence is stored at stride 128 (= S) with no
per-batch gap, so a conv tap at offset kk is one flat contiguous 512-wide
moving operand covering 4 batches; output columns t in [L, 128) accumulate
garbage that the evacuation slices away.

Perf notes (from trace analysis):
  - PE starts HAM-throttled at half clock and un-throttles only after ~5us
    of sustained high-duty matmul activity -> big junk warmup matmuls fill
    the initial DMA window (small ones don't count toward un-throttle).
  - HWDGE loads: ~0.1 B/ns per queue for >=1KB-per-partition elements, first
    packets ~1.4us after the dispatch instruction; loads are laddered across
    both queues in consumption order, with the k=4 bank first (most compute
    per embedding byte -> demand tracks supply with no stalls).
  - Stats leave as several small DMAs; the final ones are split by partition
    half across both queues since small-element DMAs are packet-rate-bound.
"""

import os
import numpy as np
import ml_dtypes

# ---------------- problem constants (hardcoded per contract) ----------------
B, S, W, V, E = 128, 128, 16, 128, 32
FILTERS = [256, 256, 256]
KS = [2, 3, 4]
NCLS = 10
EPS = 1e-5
NCORES = 8
BL = B // NCORES             # 16 batches per core
CIN = W * E                  # 512 conv input channels
NCC = CIN // 128             # 4 contraction chunks
NPAIR = NCC // 2             # 2 DoubleRow chunk pairs
LS = [S - k + 1 for k in KS]  # 127, 126, 125 valid conv positions
XH = 8 * 128                 # one batch-half (8 batches x 128) elems
XBLK = 544                   # quad block: 4 batches x 128 + 32 tap/pad slack
EMB_FREE = 8 * 2 * XBLK      # blob: quad-major (q, p, c, x) blocks = 8704
SC_A = 64.0                  # activation fp8 scale
SC_W = 64.0                  # weight fp8 scale
# group order: k=4 bank first (demand tracks DMA supply) and k=4 bank LAST:
# scalar-engine evacuation (square+accum ~0.97us/piece) exceeds a k=2 piece's
# 0.86us stream time, so k=2 groups interleave with k=3 groups (whose slack
# drains the backlog) and the stream ends on the k=4 bank whose evacuation
# drains in-line
GROUPS = [(2, 0), (0, 0), (1, 0), (0, 1), (1, 1), (2, 1)]
# per-group evacuation pieces: (stat block col, nb batches); last group ends
# with 2+1+1 batches so the tail only trails by single-batch pieces whose
# sum-of-squares comes from vector bn_stats (no scalar round-trip)
PIECES_FULL = [(0, 4), (5, 4), (10, 4), (15, 4)]
# second-to-last group: final piece at col 18 so its trailing stats share the
# [18:32] column window with the last group's and ride one fused DMA
PIECES_G4 = [(0, 4), (5, 4), (10, 4), (18, 4)]
PIECES_LAST = [(0, 4), (5, 4), (10, 4), (15, 2), (18, 1), (25, 1)]
NSTAT = 32                   # full groups use cols 0..19; last group: max 15
                             # sq 17 for (15,2); col 18 max + 19:25 bn_stats;
                             # col 25 max + 26:32 bn_stats
NWARM_SMALL = 2              # 128-col warmups
NWARM_BIG = 13               # 384-col warmups on the garbage region
# weight blob column offsets: groups laid out [g0, g1, g2, g5, g3, g4] so the
# weight transfers [g0], [g1 g2], [g5], [g3 g4] are contiguous
WOFF = [0, 2048, 3072, 6656, 7680, 4608]
F8 = ml_dtypes.float8_e4m3   # TRN FP8_EXP4: bias 7, max +-240

_CACHE = {}
_LAST_RESULTS = None


def _group_tiles(bank):
    return [(ccp, kk) for ccp in range(NPAIR) for kk in range(KS[bank])]


def _weight_tile_count():
    return sum(len(_group_tiles(bank)) for bank, _ in GROUPS)


def _build_bass(need_min):
    import concourse.tile as tile
    from concourse import bacc, mybir
    from contextlib import ExitStack

    nc = bacc.Bacc("TRN2", target_bir_lowering=False, debug=False, enable_asserts=False)

    ntiles = _weight_tile_count()  # 36 DoubleRow tiles of [128, 2, 128]
    nstat = NSTAT + (16 if need_min else 0)
    DR = mybir.MatmulPerfMode.DoubleRow
    emb_d = nc.dram_tensor(
        "emb", [128, EMB_FREE], mybir.dt.float8e4, kind="ExternalInput"
    ).ap()
    wts_d = nc.dram_tensor(
        "wts", [128, ntiles * 256], mybir.dt.float8e4, kind="ExternalInput"
    ).ap()
    stats_d = nc.dram_tensor(
        "stats", [len(GROUPS), 128, nstat], mybir.dt.float32, kind="ExternalOutput"
    ).ap()

    # ---- raw SBUF buffers + pre-TileContext loads: dma_start issued right
    # after engine init (before the TileContext entry barrier) starts the
    # HWDGE queues ~0.8us earlier than in-context loads. Completion is
    # tracked manually: each transfer incs its queue's semaphore by 16;
    # first-consumer matmuls carry wait_op(sem >= 16*k). ----
    warm_t = nc.alloc_sbuf_tensor("warm_raw", [128, 1028], mybir.dt.float8e4)
    emb_t = nc.alloc_sbuf_tensor("embr", [128, EMB_FREE], mybir.dt.float8e4)
    wt_t = nc.alloc_sbuf_tensor("wtr", [128, ntiles * 256], mybir.dt.float8e4)
    # one semaphore PER TRANSFER: a queue's 16 DMA engines start staggered
    # and interleave chunks of consecutive transfers, so a shared per-queue
    # counter hitting 16*k does NOT mean the k-th transfer finished. Few,
    # large transfers amortize the ~1.2us engine-stagger completion latency.
    SEMS = []

    def _load(eng, dst_t, c0, c1, src):
        s = nc.alloc_semaphore(f"ld{len(SEMS)}")
        SEMS.append(s)
        eng.dma_start(dst_t.ap()[:, c0:c1], src[:, c0:c1]).then_inc(s, 16)
        return (len(SEMS) - 1, 16)

    # laddered in consumption order across both queues; quad 0 is split into
    # its two pair-blocks (one per queue) so the stream starts on ~400KB, the
    # rest ride fused transfers to amortize the per-transfer stagger
    # gpsimd's SWDGE ring serves as a third load queue for the two blocks
    # with the most demand slack (p1q1, q3), shortening the HWDGE ladders
    w_g0 = _load(nc.sync, wt_t, 0, 2048, wts_d)                 # g0 (k=4)
    e_p0q0 = _load(nc.scalar, emb_t, 0, 1088, emb_d)
    e_p1q0 = _load(nc.sync, emb_t, 1088, 2176, emb_d)
    e_p0q1 = _load(nc.scalar, emb_t, 2176, 3264, emb_d)
    e_p1q1 = _load(nc.gpsimd, emb_t, 3264, 4352, emb_d)
    e_q2 = _load(nc.sync, emb_t, 4352, 6528, emb_d)
    e_q3 = _load(nc.gpsimd, emb_t, 6528, 8704, emb_d)
    w_34 = _load(nc.sync, wt_t, 6656, 9216, wts_d)              # g3, g4
    w_12 = _load(nc.scalar, wt_t, 2048, 4608, wts_d)            # g1, g2
    w_5 = _load(nc.sync, wt_t, 4608, 6656, wts_d)               # g5
    EMB_W = [
        [e_p0q0, e_p0q1, e_q2, e_q3],
        [e_p1q0, e_p1q1, e_q2, e_q3],
    ]
    WT_W = [[w_g0], [w_12], [w_12], [w_34], [w_34], [w_5]]
    pending_waits = []

    with tile.TileContext(nc) as tc, ExitStack() as ctx:
        psum_pool = ctx.enter_context(tc.tile_pool(name="psum", bufs=7, space="PSUM"))
        stats_pool = ctx.enter_context(tc.tile_pool(name="stats", bufs=3))
        scr_pool = ctx.enter_context(tc.tile_pool(name="scr", bufs=4))

        # ---- PE warmup: junk DoubleRow matmuls while input DMAs stream, so
        # HAM un-throttles before/near the real stream start (PE boots at
        # half clock; only sustained full-width activity re-ramps it, and an
        # idle gap resets the credit). Operands are uninitialized raw-SBUF
        # garbage — fp8e4 has no inf, and NaN products land in a junk PSUM
        # tile nothing reads — so the first warmup issues the moment the PE
        # enters the kernel body. ----
        warm = warm_t.ap()
        wlhs = warm[:, :256].rearrange("p (c f) -> p c f", c=2)
        wrhs_s = warm[:, :256].rearrange("p (c x) -> p c x", c=2)     # x=128
        wrhs_b = warm[:, 260:1028].rearrange("p (c x) -> p c x", c=2)  # x=384
        wps = psum_pool.tile(
            [128, 384], mybir.dt.float32, tag="warm", bufs=1, name="wps"
        )
        for _ in range(NWARM_SMALL):
            nc.tensor.matmul(
                wps[:, :128], wlhs, wrhs_s, start=True, stop=True, perf_mode=DR
            )
        for _ in range(NWARM_BIG):
            nc.tensor.matmul(
                wps[:], wlhs, wrhs_b, start=True, stop=True, perf_mode=DR
            )

        def rhs_ap(ccp, q, kk, boff, nb):
            # moving operand [128, 2, nb*128]: dim1 steps between the two
            # chunks of the pair, free covers nb batches at stride 128
            off = (2 * q + ccp) * 2 * XBLK
            src = emb_t.ap()[:, off : off + 2 * XBLK].rearrange(
                "p (c x) -> p c x", c=2
            )
            x0 = boff * 128 + kk
            return src[:, :, x0 : x0 + nb * 128]

        # ---- conv banks: quad-major so each accumulation group stops early
        # and its evacuation overlaps the next quad's matmul stream. The last
        # group's final 2+1+1 batches run at the very end so only
        # single-batch evacuations + small split stats DMAs trail. ----
        # the last two groups share one tile so their trailing [18:32] stats
        # can leave as a single fused strided DMA pair
        st45 = stats_pool.tile(
            [128, 2 * nstat], mybir.dt.float32, tag="st45", bufs=1, name="st45"
        )
        st_sb = [
            stats_pool.tile([128, nstat], mybir.dt.float32, tag="st", name=f"st{g}")
            for g in range(len(GROUPS) - 2)
        ] + [st45[:, 0:nstat], st45[:, nstat : 2 * nstat]]

        def merged_waits(wlists):
            best = {}
            for sem_idx, val in wlists:
                best[sem_idx] = max(best.get(sem_idx, 0), val)
            return list(best.items())

        def emit_mms(g, bank, bidx, nb, pi):
            tiles = _group_tiles(bank)
            wt = wt_t.ap()
            q = bidx // 4
            boff = bidx - q * 4
            half = len(tiles) // 2
            w0 = merged_waits([EMB_W[0][q]] + WT_W[g][:1])
            w1 = merged_waits([EMB_W[1][q]] + WT_W[g][1:])
            ps = psum_pool.tile(
                [128, nb * 128], mybir.dt.float32, tag="ps", name=f"ps{g}_{pi}"
            )
            for i, (ccp, kk) in enumerate(tiles):
                lhs = wt[:, WOFF[g] + i * 256 : WOFF[g] + (i + 1) * 256].rearrange(
                    "p (c f) -> p c f", c=2
                )
                inst = nc.tensor.matmul(
                    ps[:], lhs, rhs_ap(ccp, q, kk, boff, nb),
                    start=(i == 0), stop=(i == len(tiles) - 1), perf_mode=DR,
                )
                # manual raw-load gating: pre-context DMAs are invisible to
                # the tile dependency tracker (and its deadlock-checking
                # simulator), so stash the waits and attach them after the
                # context is scheduled
                for sem_idx, val in (w0 if i == 0 else w1 if i == half else []):
                    pending_waits.append((inst, sem_idx, val))
            L = LS[bank]
            return ps[:].rearrange("p (b t) -> p b t", t=128)[:, :, :L]

        def evac_max_sq(g, col, nb, bidx, pv, L):
            st = st_sb[g]
            nc.vector.tensor_reduce(
                st[:, col : col + nb], pv, axis=mybir.AxisListType.X,
                op=mybir.AluOpType.max,
            )
            if need_min:
                nc.vector.tensor_reduce(
                    st[:, NSTAT + bidx : NSTAT + bidx + nb], pv,
                    axis=mybir.AxisListType.X, op=mybir.AluOpType.min,
                )
            scr = scr_pool.tile([128, 512], mybir.dt.float32, tag="scr")
            scr_v = scr[:, : nb * L].rearrange("p (b t) -> p b t", t=L)
            nc.scalar.activation(
                scr_v, pv,
                mybir.ActivationFunctionType.Square,
                accum_out=st[:, col + nb : col + nb + 1],
            )

        def emit_piece(g, bank, col, nb, bidx, pi):
            pv = emit_mms(g, bank, bidx, nb, pi)
            evac_max_sq(g, col, nb, bidx, pv, LS[bank])

        def emit_group_pieces(g, pieces, bidx0):
            bank, _ = GROUPS[g]
            bidx = bidx0
            for pi, (col, nb) in enumerate(pieces):
                emit_piece(g, bank, col, nb, bidx, f"{pi}_{bidx}")
                bidx += nb

        def dma_stats(eng, g, c0, c1):
            eng.dma_start(stats_d[g][:, c0:c1], st_sb[g][:, c0:c1])

        def dma_stats_split(g, c0, c1):
            # partition-halved across two queues: small-elem DMAs are
            # packet-rate-bound, so two halves finish ~2x sooner. Stats ride
            # sync/gpsimd only — a dispatch costs ~0.6us of ENGINE time, and
            # the scalar engine is the evacuation bottleneck
            nc.sync.dma_start(stats_d[g][0:64, c0:c1], st_sb[g][0:64, c0:c1])
            nc.gpsimd.dma_start(stats_d[g][64:128, c0:c1], st_sb[g][64:128, c0:c1])

        glast = len(GROUPS) - 1
        for g in range(glast - 1):
            emit_group_pieces(g, PIECES_FULL, 0)
            dma_stats(nc.sync if g % 2 == 0 else nc.gpsimd, g, 0, 20)
            if need_min:
                dma_stats(nc.sync if g % 2 == 0 else nc.gpsimd, g, NSTAT, NSTAT + 16)
        # last group's first 14 batches run before the second-to-last group,
        # so their evacuations overlap that group's stream
        bank5 = GROUPS[glast][0]
        L5 = LS[bank5]
        for pi, (col, nb) in enumerate(PIECES_LAST[:4]):
            emit_piece(glast, bank5, col, nb, sum(p[1] for p in PIECES_LAST[:pi]),
                       f"l{pi}")
        dma_stats(nc.gpsimd, glast, 0, 18)
        if need_min:
            dma_stats(nc.gpsimd, glast, NSTAT, NSTAT + 14)
        emit_group_pieces(glast - 1, PIECES_G4, 0)
        dma_stats_split(glast - 1, 0, 15)
        if need_min:
            dma_stats(nc.sync, glast - 1, NSTAT, NSTAT + 16)

        # trailing single-batch pieces: max on gpsimd/vector, sum of squares
        # via vector bn_stats -- no scalar-engine round trip at the tail
        st5 = st_sb[glast]
        pv14 = emit_mms(glast, bank5, 14, 1, "l4")
        nc.vector.tensor_reduce(
            st5[:, 18:19], pv14, axis=mybir.AxisListType.X, op=mybir.AluOpType.max
        )
        nc.vector.bn_stats(st5[:, 19:25], pv14[:, 0, :])
        if need_min:
            nc.vector.tensor_reduce(
                st5[:, NSTAT + 14 : NSTAT + 15], pv14,
                axis=mybir.AxisListType.X, op=mybir.AluOpType.min,
            )
        pv15 = emit_mms(glast, bank5, 15, 1, "l5")
        nc.vector.tensor_reduce(
            st5[:, 25:26], pv15, axis=mybir.AxisListType.X, op=mybir.AluOpType.max
        )
        nc.vector.bn_stats(st5[:, 26:32], pv15[:, 0, :])
        if need_min:
            nc.vector.tensor_reduce(
                st5[:, NSTAT + 15 : NSTAT + 16], pv15,
                axis=mybir.AxisListType.X, op=mybir.AluOpType.min,
            )
        # final fused stats: one strided DMA pair carries the [18:32] window
        # of BOTH trailing groups (they share the st45 tile)
        src45 = st45[:].rearrange("p (g c) -> p g c", g=2)[:, :, 18:32]
        dst45 = stats_d[glast - 1 : glast + 1, :, 18:32].rearrange(
            "g p c -> p g c"
        )
        nc.sync.dma_start(dst45[0:64], src45[0:64])
        nc.scalar.dma_start(dst45[64:128], src45[64:128])
        if need_min:
            dma_stats(nc.gpsimd, glast, NSTAT + 14, nstat)

    # attach the raw-load gating waits now that tile scheduling is done;
    # check=False since Bacc's generate_event_semaphores pass legalizes
    # instructions carrying more than one wait
    for inst, sem_idx, val in pending_waits:
        inst.wait_op(SEMS[sem_idx], val, "sem-ge", check=False)

    nc.compile()
    return nc


def _get_compiled(need_min):
    key = ("nc", need_min)
    if key not in _CACHE:
        _CACHE[key] = _build_bass(need_min)
    return _CACHE[key]


def _maybe_enable_trace():
    if os.environ.get("KERNEL_TRACE") != "1":
        return False
    try:
        import sys, types

        if "antenv.axon_hooks" not in sys.modules:
            mod = types.ModuleType("antenv.axon_hooks")
            _h = {"hook": None}
            mod.set_axon_ntff_profile_hook = lambda h: _h.__setitem__("hook", h)
            mod.get_axon_ntff_profile_hook = lambda: _h["hook"]
            sys.modules["antenv.axon_hooks"] = mod
            import antenv

            antenv.axon_hooks = mod
            from trn_agent_boot.trn_boot import _ntff_profile_via_ctypes

            mod.set_axon_ntff_profile_hook(
                _ntff_profile_via_ctypes("/opt/axon/libaxon_pjrt.so")
            )
        import concourse.bass_utils as bu

        bu.upload_artifacts = lambda tmpdir: tmpdir
        return True
    except Exception:
        return False


def _q8(a, sc):
    return np.clip(np.asarray(a, dtype=np.float32) * sc, -240.0, 240.0).astype(F8)


def kernel(
    x, emb_w,
    conv_w0, conv_b0, bn_g0, bn_b0,
    conv_w1, conv_b1, bn_g1, bn_b1,
    conv_w2, conv_b2, bn_g2, bn_b2,
    fc1_w, fc1_b, bn1_g, bn1_b, fc2_w, fc2_b,
):
    global _LAST_RESULTS
    from concourse.bass_utils import run_bass_kernel_spmd

    x = np.asarray(x, dtype=np.float32)
    emb_w = np.asarray(emb_w, dtype=np.float32)
    conv_ws = [np.asarray(w, dtype=np.float32) for w in (conv_w0, conv_w1, conv_w2)]
    bn_gs = [np.asarray(v, dtype=np.float64) for v in (bn_g0, bn_g1, bn_g2)]
    bn_bs = [np.asarray(v, dtype=np.float64) for v in (bn_b0, bn_b1, bn_b2)]
    need_min = bool((np.concatenate(bn_gs) < 0.0).any())

    # ---- host: embedding (x is one-hot in practice; dense matmul is exact) ----
    e = x.reshape(-1, V) @ emb_w                       # [B*S*W, E]
    e = e.reshape(B, S, CIN)                           # [B, S, 512]
    embT = np.ascontiguousarray(e.transpose(2, 0, 1))  # [512, B, S]
    emb8 = _q8(embT, SC_A)                             # [512, B, 128]

    # ---- pack device inputs (weights at blob offsets WOFF) ----
    ntiles = _weight_tile_count()
    wts = np.empty((128, ntiles * 256), dtype=F8)
    for g, (bank, fc) in enumerate(GROUPS):
        cwq = _q8(conv_ws[bank], SC_W)                 # [256, 512, k]
        i = WOFF[g] // 256
        for ccp, kk in _group_tiles(bank):
            blk = cwq[fc * 128 : (fc + 1) * 128,
                      2 * ccp * 128 : (2 * ccp + 2) * 128, kk]  # [f, 2*128]
            # target [p, c*128 + f] = blk[f, c*128 + p]
            wts[:, i * 256 : (i + 1) * 256] = (
                blk.reshape(128, 2, 128).transpose(2, 1, 0).reshape(128, 256)
            )
            i += 1

    # emb8 viewed [pair, c, p, batch, t]
    ev = emb8.reshape(NPAIR, 2, 128, B, S)
    in_maps = []
    for c in range(NCORES):
        v = ev[:, :, :, c * BL : (c + 1) * BL, :].reshape(NPAIR, 2, 128, 2, 8, S)
        # [pair, c2, p, h, b, t] -> [pair, p, h, c2, (b t)] half-streams
        hs = v.transpose(0, 2, 3, 1, 4, 5).reshape(NPAIR, 128, 2, 2, XH)
        # quad blocks: q = h*2+qq covers x [512qq, 512qq+544) of half h (zero
        # padded); blob is quad-major [q, pair, c, x] so one transfer carries
        # a full quad (both pairs) as contiguous 2176B per partition
        tmp = np.zeros((128, 2, 2, NPAIR, 2, XBLK), dtype=F8)  # [p,h,qq,pair,c,x]
        hsp = hs.transpose(1, 2, 0, 3, 4)  # [p, h, pair, c2, XH]
        tmp[:, :, 0, :, :, 0:XBLK] = hsp[..., 0:XBLK]
        tmp[:, :, 1, :, :, 0 : XH - 512] = hsp[..., 512:XH]
        in_maps.append({"emb": tmp.reshape(128, EMB_FREE), "wts": wts})

    nc = _get_compiled(need_min)
    trace = _maybe_enable_trace()
    res = run_bass_kernel_spmd(
        nc, in_maps, core_ids=list(range(NCORES)), trace=trace,
        tmpdir=os.environ.get("KERNEL_TRACE_DIR") or None,
    )
    _LAST_RESULTS = res

    # ---- host: combine stats -> BN -> pooled -> fc head (float64) ----
    FT = sum(FILTERS)  # 768
    inv = 1.0 / (SC_A * SC_W)
    cmax = np.empty((FT, B), dtype=np.float64)
    cmin = np.empty((FT, B), dtype=np.float64) if need_min else None
    sumsq = np.zeros(FT, dtype=np.float64)

    def bn_sumsq(bn):
        # bn_stats output: [cnt, mean, cnt*var] x (even, odd) half-streams
        return (bn[:, 2] + bn[:, 0] * bn[:, 1] ** 2
                + bn[:, 5] + bn[:, 3] * bn[:, 4] ** 2)

    for c in range(NCORES):
        stats = res.results[c]["stats"].astype(np.float64)  # [6, 128, nstat]
        for g, (bank, fc) in enumerate(GROUPS):
            ch = bank * 256 + fc * 128
            sl = slice(ch, ch + 128)
            pieces = (PIECES_LAST if g == len(GROUPS) - 1
                      else PIECES_G4 if g == len(GROUPS) - 2 else PIECES_FULL)
            bidx = 0
            for col, nb in pieces:
                bs = slice(c * BL + bidx, c * BL + bidx + nb)
                cmax[sl, bs] = stats[g, :, col : col + nb] * inv
                if g == len(GROUPS) - 1 and nb == 1:
                    sumsq[sl] += bn_sumsq(stats[g, :, col + 1 : col + 7]) * inv * inv
                else:
                    sumsq[sl] += stats[g, :, col + nb] * inv * inv
                if need_min:
                    cmin[sl, bs] = stats[g, :, NSTAT + bidx : NSTAT + bidx + nb] * inv
                bidx += nb

    # channel means via the factorized sum (exact: sum_t conv = w . window-sums)
    embT64 = embT.astype(np.float64)
    st_sum = embT64.sum(axis=1)                        # [512, S] summed over batch
    cum = np.concatenate(
        [np.zeros((CIN, 1)), np.cumsum(st_sum, axis=1)], axis=1
    )                                                  # [512, S+1]
    mean = np.empty(FT, dtype=np.float64)
    for bank in range(3):
        k, L = KS[bank], LS[bank]
        cw = conv_ws[bank].astype(np.float64)          # [256, 512, k]
        hs = np.stack([cum[:, kk + L] - cum[:, kk] for kk in range(k)], axis=1)
        mean[bank * 256 : (bank + 1) * 256] = (
            np.einsum("fck,ck->f", cw, hs) / (B * L)
        )

    counts = np.repeat([B * L for L in LS], FILTERS)
    var = sumsq / counts - mean * mean
    g_all = np.concatenate(bn_gs)
    b_all = np.concatenate(bn_bs)
    s = g_all / np.sqrt(var + EPS)
    shift = b_all - mean * s
    M = np.where(s[:, None] >= 0.0, cmax, cmin if need_min else cmax)  # [768, B]
    pooled = np.maximum(s[:, None] * M + shift[:, None], 0.0).T  # [B, 768]

    z = pooled @ np.asarray(fc1_w, dtype=np.float64) + np.asarray(
        fc1_b, dtype=np.float64
    )
    mu = z.mean(axis=0, keepdims=True)
    vz = np.square(z - mu).mean(axis=0, keepdims=True)
    z = (z - mu) / np.sqrt(vz + EPS) * np.asarray(
        bn1_g, dtype=np.float64
    ) + np.asarray(bn1_b, dtype=np.float64)
    z = np.maximum(z, 0.0)
    logits = z @ np.asarray(fc2_w, dtype=np.float64) + np.asarray(
        fc2_b, dtype=np.float64
    )
    logits -= logits.max(axis=1, keepdims=True)
    p = np.exp(logits)
    p /= p.sum(axis=1, keepdims=True)
    return p.astype(np.float32)


# revision 58
# speedup vs baseline: 1.0031x; 1.0031x over previous
"""Trainium2 Bass kernel for the char-CNN NLP model (data-parallel over 8 cores).

Pipeline:
  host:   emb = x @ emb_w (one-hot projection), laid out [cin, batch, seq],
          quantized to fp8e4 (scaled x64; TRN FP8_EXP4 == ml_dtypes.float8_e4m3)
  device: 3 parallel 1-D conv banks (k=2,3,4; 256 filters each) as fp8
          DoubleRow matmuls (two cin-chunks contracted per pass, fp32 PSUM);
          per (channel, batch) max over sequence; per channel sum of squares
          -> tiny stats tensor per core
  host:   batchnorm statistics from the factorized mean + device sumsq,
          monotone-affine BN+ReLU+maxpool reconstruction from max (min when
          some bn gamma < 0), fc1 -> bn -> relu -> fc2 -> softmax

BN(c+bias) is affine per channel, so max_t relu(bn(c)) = relu(s*M + t) with
M = max_t c if s>=0 else min_t c - exact, and the conv bias cancels inside BN.

Layout trick: each batch's sequence is stored at stride 128 (= S) with no
per-batch gap, so a conv tap at offset kk is one flat contiguous 512-wide
moving operand covering 4 batches; output columns t in [L, 128) accumulate
garbage that the evacuation slices away.

Perf notes (from trace analysis):
  - PE starts HAM-throttled at half clock and un-throttles only after ~5us
    of sustained high-duty matmul activity -> big junk warmup matmuls fill
    the initial DMA window (small ones don't count toward un-throttle).
  - HWDGE loads: ~0.1 B/ns per queue for >=1KB-per-partition elements, first
    packets ~1.4us after the dispatch instruction; loads are laddered across
    both queues in consumption order, with the k=4 bank first (most compute
    per embedding byte -> demand tracks supply with no stalls).
  - Stats leave as several small DMAs; the final ones are split by partition
    half across both queues since small-element DMAs are packet-rate-bound.
"""

import os
import numpy as np
import ml_dtypes

# ---------------- problem constants (hardcoded per contract) ----------------
B, S, W, V, E = 128, 128, 16, 128, 32
FILTERS = [256, 256, 256]
KS = [2, 3, 4]
NCLS = 10
EPS = 1e-5
NCORES = 8
BL = B // NCORES             # 16 batches per core
CIN = W * E                  # 512 conv input channels
NCC = CIN // 128             # 4 contraction chunks
NPAIR = NCC // 2             # 2 DoubleRow chunk pairs
LS = [S - k + 1 for k in KS]  # 127, 126, 125 valid conv positions
XH = 8 * 128                 # one batch-half (8 batches x 128) elems
XBLK = 544                   # quad block: 4 batches x 128 + 32 tap/pad slack
EMB_FREE = 8 * 2 * XBLK      # blob: quad-major (q, p, c, x) blocks = 8704
SC_A = 64.0                  # activation fp8 scale
SC_W = 64.0                  # weight fp8 scale
# group order: k=4 bank first (demand tracks DMA supply) and k=4 bank LAST:
# scalar-engine evacuation (square+accum ~0.97us/piece) exceeds a k=2 piece's
# 0.86us stream time, so k=2 groups interleave with k=3 groups (whose slack
# drains the backlog) and the stream ends on the k=4 bank whose evacuation
# drains in-line
GROUPS = [(2, 0), (0, 0), (1, 0), (0, 1), (1, 1), (2, 1)]
# per-group evacuation pieces: (stat block col, nb batches); last group ends
# with 2+1+1 batches so the tail only trails by single-batch pieces whose
# sum-of-squares comes from vector bn_stats (no scalar round-trip)
PIECES_FULL = [(0, 4), (5, 4), (10, 4), (15, 4)]
# second-to-last group: final piece at col 18 so its trailing stats share the
# [18:32] column window with the last group's and ride one fused DMA
PIECES_G4 = [(0, 4), (5, 4), (10, 4), (18, 4)]
PIECES_LAST = [(0, 4), (5, 4), (10, 4), (15, 2), (18, 1), (25, 1)]
NSTAT = 32                   # full groups use cols 0..19; last group: max 15
                             # sq 17 for (15,2); col 18 max + 19:25 bn_stats;
                             # col 25 max + 26:32 bn_stats
NWARM_SMALL = 2              # 128-col warmups
NWARM_BIG = 13               # 384-col warmups on the garbage region
# weight blob column offsets: groups laid out [g0, g1, g2, g5, g3, g4] so the
# weight transfers [g0], [g1 g2], [g5], [g3 g4] are contiguous
WOFF = [0, 2048, 3072, 6656, 7680, 4608]
F8 = ml_dtypes.float8_e4m3   # TRN FP8_EXP4: bias 7, max +-240

_CACHE = {}
_LAST_RESULTS = None


def _group_tiles(bank):
    return [(ccp, kk) for ccp in range(NPAIR) for kk in range(KS[bank])]


def _weight_tile_count():
    return sum(len(_group_tiles(bank)) for bank, _ in GROUPS)


def _build_bass(need_min):
    import concourse.tile as tile
    from concourse import bacc, mybir
    from contextlib import ExitStack

    nc = bacc.Bacc("TRN2", target_bir_lowering=False, debug=False, enable_asserts=False)

    ntiles = _weight_tile_count()  # 36 DoubleRow tiles of [128, 2, 128]
    nstat = NSTAT + (16 if need_min else 0)
    DR = mybir.MatmulPerfMode.DoubleRow
    emb_d = nc.dram_tensor(
        "emb", [128, EMB_FREE], mybir.dt.float8e4, kind="ExternalInput"
    ).ap()
    wts_d = nc.dram_tensor(
        "wts", [128, ntiles * 256], mybir.dt.float8e4, kind="ExternalInput"
    ).ap()
    stats_d = nc.dram_tensor(
        "stats", [len(GROUPS), 128, nstat], mybir.dt.float32, kind="ExternalOutput"
    ).ap()

    # ---- raw SBUF buffers + pre-TileContext loads: dma_start issued right
    # after engine init (before the TileContext entry barrier) starts the
    # HWDGE queues ~0.8us earlier than in-context loads. Completion is
    # tracked manually: each transfer incs its queue's semaphore by 16;
    # first-consumer matmuls carry wait_op(sem >= 16*k). ----
    warm_t = nc.alloc_sbuf_tensor("warm_raw", [128, 1028], mybir.dt.float8e4)
    emb_t = nc.alloc_sbuf_tensor("embr", [128, EMB_FREE], mybir.dt.float8e4)
    wt_t = nc.alloc_sbuf_tensor("wtr", [128, ntiles * 256], mybir.dt.float8e4)
    # one semaphore PER TRANSFER: a queue's 16 DMA engines start staggered
    # and interleave chunks of consecutive transfers, so a shared per-queue
    # counter hitting 16*k does NOT mean the k-th transfer finished. Few,
    # large transfers amortize the ~1.2us engine-stagger completion latency.
    SEMS = []

    def _load(eng, dst_t, c0, c1, src):
        s = nc.alloc_semaphore(f"ld{len(SEMS)}")
        SEMS.append(s)
        eng.dma_start(dst_t.ap()[:, c0:c1], src[:, c0:c1]).then_inc(s, 16)
        return (len(SEMS) - 1, 16)

    # laddered in consumption order across both queues; quad 0 is split into
    # its two pair-blocks (one per queue) so the stream starts on ~400KB, the
    # rest ride fused transfers to amortize the per-transfer stagger
    # gpsimd's SWDGE ring serves as a third load queue for the two blocks
    # with the most demand slack (p1q1, q3), shortening the HWDGE ladders
    w_g0 = _load(nc.sync, wt_t, 0, 2048, wts_d)                 # g0 (k=4)
    e_p0q0 = _load(nc.scalar, emb_t, 0, 1088, emb_d)
    e_p1q0 = _load(nc.sync, emb_t, 1088, 2176, emb_d)
    e_p0q1 = _load(nc.gpsimd, emb_t, 2176, 3264, emb_d)
    e_p1q1 = _load(nc.gpsimd, emb_t, 3264, 4352, emb_d)
    e_q2 = _load(nc.sync, emb_t, 4352, 6528, emb_d)
    e_q3 = _load(nc.gpsimd, emb_t, 6528, 8704, emb_d)
    w_34 = _load(nc.sync, wt_t, 6656, 9216, wts_d)              # g3, g4
    w_12 = _load(nc.scalar, wt_t, 2048, 4608, wts_d)            # g1, g2
    w_5 = _load(nc.sync, wt_t, 4608, 6656, wts_d)               # g5
    EMB_W = [
        [e_p0q0, e_p0q1, e_q2, e_q3],
        [e_p1q0, e_p1q1, e_q2, e_q3],
    ]
    WT_W = [[w_g0], [w_12], [w_12], [w_34], [w_34], [w_5]]
    pending_waits = []

    with tile.TileContext(nc) as tc, ExitStack() as ctx:
        psum_pool = ctx.enter_context(tc.tile_pool(name="psum", bufs=7, space="PSUM"))
        stats_pool = ctx.enter_context(tc.tile_pool(name="stats", bufs=3))
        scr_pool = ctx.enter_context(tc.tile_pool(name="scr", bufs=4))

        # ---- PE warmup: junk DoubleRow matmuls while input DMAs stream, so
        # HAM un-throttles before/near the real stream start (PE boots at
        # half clock; only sustained full-width activity re-ramps it, and an
        # idle gap resets the credit). Operands are uninitialized raw-SBUF
        # garbage — fp8e4 has no inf, and NaN products land in a junk PSUM
        # tile nothing reads — so the first warmup issues the moment the PE
        # enters the kernel body. ----
        warm = warm_t.ap()
        wlhs = warm[:, :256].rearrange("p (c f) -> p c f", c=2)
        wrhs_s = warm[:, :256].rearrange("p (c x) -> p c x", c=2)     # x=128
        wrhs_b = warm[:, 260:1028].rearrange("p (c x) -> p c x", c=2)  # x=384
        wps = psum_pool.tile(
            [128, 384], mybir.dt.float32, tag="warm", bufs=1, name="wps"
        )
        for _ in range(NWARM_SMALL):
            nc.tensor.matmul(
                wps[:, :128], wlhs, wrhs_s, start=True, stop=True, perf_mode=DR
            )
        for _ in range(NWARM_BIG):
            nc.tensor.matmul(
                wps[:], wlhs, wrhs_b, start=True, stop=True, perf_mode=DR
            )

        def rhs_ap(ccp, q, kk, boff, nb):
            # moving operand [128, 2, nb*128]: dim1 steps between the two
            # chunks of the pair, free covers nb batches at stride 128
            off = (2 * q + ccp) * 2 * XBLK
            src = emb_t.ap()[:, off : off + 2 * XBLK].rearrange(
                "p (c x) -> p c x", c=2
            )
            x0 = boff * 128 + kk
            return src[:, :, x0 : x0 + nb * 128]

        # ---- conv banks: quad-major so each accumulation group stops early
        # and its evacuation overlaps the next quad's matmul stream. The last
        # group's final 2+1+1 batches run at the very end so only
        # single-batch evacuations + small split stats DMAs trail. ----
        # the last two groups share one tile so their trailing [18:32] stats
        # can leave as a single fused strided DMA pair
        st45 = stats_pool.tile(
            [128, 2 * nstat], mybir.dt.float32, tag="st45", bufs=1, name="st45"
        )
        st_sb = [
            stats_pool.tile([128, nstat], mybir.dt.float32, tag="st", name=f"st{g}")
            for g in range(len(GROUPS) - 2)
        ] + [st45[:, 0:nstat], st45[:, nstat : 2 * nstat]]

        def merged_waits(wlists):
            best = {}
            for sem_idx, val in wlists:
                best[sem_idx] = max(best.get(sem_idx, 0), val)
            return list(best.items())

        def emit_mms(g, bank, bidx, nb, pi):
            tiles = _group_tiles(bank)
            wt = wt_t.ap()
            q = bidx // 4
            boff = bidx - q * 4
            half = len(tiles) // 2
            w0 = merged_waits([EMB_W[0][q]] + WT_W[g][:1])
            w1 = merged_waits([EMB_W[1][q]] + WT_W[g][1:])
            ps = psum_pool.tile(
                [128, nb * 128], mybir.dt.float32, tag="ps", name=f"ps{g}_{pi}"
            )
            for i, (ccp, kk) in enumerate(tiles):
                lhs = wt[:, WOFF[g] + i * 256 : WOFF[g] + (i + 1) * 256].rearrange(
                    "p (c f) -> p c f", c=2
                )
                inst = nc.tensor.matmul(
                    ps[:], lhs, rhs_ap(ccp, q, kk, boff, nb),
                    start=(i == 0), stop=(i == len(tiles) - 1), perf_mode=DR,
                )
                # manual raw-load gating: pre-context DMAs are invisible to
                # the tile dependency tracker (and its deadlock-checking
                # simulator), so stash the waits and attach them after the
                # context is scheduled
                for sem_idx, val in (w0 if i == 0 else w1 if i == half else []):
                    pending_waits.append((inst, sem_idx, val))
            L = LS[bank]
            return ps[:].rearrange("p (b t) -> p b t", t=128)[:, :, :L]

        def evac_max_sq(g, col, nb, bidx, pv, L):
            st = st_sb[g]
            nc.vector.tensor_reduce(
                st[:, col : col + nb], pv, axis=mybir.AxisListType.X,
                op=mybir.AluOpType.max,
            )
            if need_min:
                nc.vector.tensor_reduce(
                    st[:, NSTAT + bidx : NSTAT + bidx + nb], pv,
                    axis=mybir.AxisListType.X, op=mybir.AluOpType.min,
                )
            scr = scr_pool.tile([128, 512], mybir.dt.float32, tag="scr")
            scr_v = scr[:, : nb * L].rearrange("p (b t) -> p b t", t=L)
            nc.scalar.activation(
                scr_v, pv,
                mybir.ActivationFunctionType.Square,
                accum_out=st[:, col + nb : col + nb + 1],
            )

        def emit_piece(g, bank, col, nb, bidx, pi):
            pv = emit_mms(g, bank, bidx, nb, pi)
            evac_max_sq(g, col, nb, bidx, pv, LS[bank])

        def emit_group_pieces(g, pieces, bidx0):
            bank, _ = GROUPS[g]
            bidx = bidx0
            for pi, (col, nb) in enumerate(pieces):
                emit_piece(g, bank, col, nb, bidx, f"{pi}_{bidx}")
                bidx += nb

        def dma_stats(eng, g, c0, c1):
            eng.dma_start(stats_d[g][:, c0:c1], st_sb[g][:, c0:c1])

        def dma_stats_split(g, c0, c1):
            # partition-halved across two queues: small-elem DMAs are
            # packet-rate-bound, so two halves finish ~2x sooner. Stats ride
            # sync/gpsimd only — a dispatch costs ~0.6us of ENGINE time, and
            # the scalar engine is the evacuation bottleneck
            nc.sync.dma_start(stats_d[g][0:64, c0:c1], st_sb[g][0:64, c0:c1])
            nc.gpsimd.dma_start(stats_d[g][64:128, c0:c1], st_sb[g][64:128, c0:c1])

        glast = len(GROUPS) - 1
        for g in range(glast - 1):
            emit_group_pieces(g, PIECES_FULL, 0)
            dma_stats(nc.sync if g % 2 == 0 else nc.gpsimd, g, 0, 20)
            if need_min:
                dma_stats(nc.sync if g % 2 == 0 else nc.gpsimd, g, NSTAT, NSTAT + 16)
        # last group's first 14 batches run before the second-to-last group,
        # so their evacuations overlap that group's stream
        bank5 = GROUPS[glast][0]
        L5 = LS[bank5]
        for pi, (col, nb) in enumerate(PIECES_LAST[:4]):
            emit_piece(glast, bank5, col, nb, sum(p[1] for p in PIECES_LAST[:pi]),
                       f"l{pi}")
        dma_stats(nc.gpsimd, glast, 0, 18)
        if need_min:
            dma_stats(nc.gpsimd, glast, NSTAT, NSTAT + 14)
        emit_group_pieces(glast - 1, PIECES_G4, 0)
        dma_stats_split(glast - 1, 0, 15)
        if need_min:
            dma_stats(nc.sync, glast - 1, NSTAT, NSTAT + 16)

        # trailing single-batch pieces: max on gpsimd/vector, sum of squares
        # via vector bn_stats -- no scalar-engine round trip at the tail
        st5 = st_sb[glast]
        pv14 = emit_mms(glast, bank5, 14, 1, "l4")
        nc.vector.tensor_reduce(
            st5[:, 18:19], pv14, axis=mybir.AxisListType.X, op=mybir.AluOpType.max
        )
        nc.vector.bn_stats(st5[:, 19:25], pv14[:, 0, :])
        if need_min:
            nc.vector.tensor_reduce(
                st5[:, NSTAT + 14 : NSTAT + 15], pv14,
                axis=mybir.AxisListType.X, op=mybir.AluOpType.min,
            )
        pv15 = emit_mms(glast, bank5, 15, 1, "l5")
        nc.vector.tensor_reduce(
            st5[:, 25:26], pv15, axis=mybir.AxisListType.X, op=mybir.AluOpType.max
        )
        nc.vector.bn_stats(st5[:, 26:32], pv15[:, 0, :])
        if need_min:
            nc.vector.tensor_reduce(
                st5[:, NSTAT + 15 : NSTAT + 16], pv15,
                axis=mybir.AxisListType.X, op=mybir.AluOpType.min,
            )
        # final fused stats: one strided DMA pair carries the [18:32] window
        # of BOTH trailing groups (they share the st45 tile)
        src45 = st45[:].rearrange("p (g c) -> p g c", g=2)[:, :, 18:32]
        dst45 = stats_d[glast - 1 : glast + 1, :, 18:32].rearrange(
            "g p c -> p g c"
        )
        nc.sync.dma_start(dst45[0:64], src45[0:64])
        nc.scalar.dma_start(dst45[64:128], src45[64:128])
        if need_min:
            dma_stats(nc.gpsimd, glast, NSTAT + 14, nstat)

    # attach the raw-load gating waits now that tile scheduling is done;
    # check=False since Bacc's generate_event_semaphores pass legalizes
    # instructions carrying more than one wait
    for inst, sem_idx, val in pending_waits:
        inst.wait_op(SEMS[sem_idx], val, "sem-ge", check=False)

    nc.compile()
    return nc


def _get_compiled(need_min):
    key = ("nc", need_min)
    if key not in _CACHE:
        _CACHE[key] = _build_bass(need_min)
    return _CACHE[key]


def _maybe_enable_trace():
    if os.environ.get("KERNEL_TRACE") != "1":
        return False
    try:
        import sys, types

        if "antenv.axon_hooks" not in sys.modules:
            mod = types.ModuleType("antenv.axon_hooks")
            _h = {"hook": None}
            mod.set_axon_ntff_profile_hook = lambda h: _h.__setitem__("hook", h)
            mod.get_axon_ntff_profile_hook = lambda: _h["hook"]
            sys.modules["antenv.axon_hooks"] = mod
            import antenv

            antenv.axon_hooks = mod
            from trn_agent_boot.trn_boot import _ntff_profile_via_ctypes

            mod.set_axon_ntff_profile_hook(
                _ntff_profile_via_ctypes("/opt/axon/libaxon_pjrt.so")
            )
        import concourse.bass_utils as bu

        bu.upload_artifacts = lambda tmpdir: tmpdir
        return True
    except Exception:
        return False


def _q8(a, sc):
    return np.clip(np.asarray(a, dtype=np.float32) * sc, -240.0, 240.0).astype(F8)


def kernel(
    x, emb_w,
    conv_w0, conv_b0, bn_g0, bn_b0,
    conv_w1, conv_b1, bn_g1, bn_b1,
    conv_w2, conv_b2, bn_g2, bn_b2,
    fc1_w, fc1_b, bn1_g, bn1_b, fc2_w, fc2_b,
):
    global _LAST_RESULTS
    from concourse.bass_utils import run_bass_kernel_spmd

    x = np.asarray(x, dtype=np.float32)
    emb_w = np.asarray(emb_w, dtype=np.float32)
    conv_ws = [np.asarray(w, dtype=np.float32) for w in (conv_w0, conv_w1, conv_w2)]
    bn_gs = [np.asarray(v, dtype=np.float64) for v in (bn_g0, bn_g1, bn_g2)]
    bn_bs = [np.asarray(v, dtype=np.float64) for v in (bn_b0, bn_b1, bn_b2)]
    need_min = bool((np.concatenate(bn_gs) < 0.0).any())

    # ---- host: embedding (x is one-hot in practice; dense matmul is exact) ----
    e = x.reshape(-1, V) @ emb_w                       # [B*S*W, E]
    e = e.reshape(B, S, CIN)                           # [B, S, 512]
    embT = np.ascontiguousarray(e.transpose(2, 0, 1))  # [512, B, S]
    emb8 = _q8(embT, SC_A)                             # [512, B, 128]

    # ---- pack device inputs (weights at blob offsets WOFF) ----
    ntiles = _weight_tile_count()
    wts = np.empty((128, ntiles * 256), dtype=F8)
    for g, (bank, fc) in enumerate(GROUPS):
        cwq = _q8(conv_ws[bank], SC_W)                 # [256, 512, k]
        i = WOFF[g] // 256
        for ccp, kk in _group_tiles(bank):
            blk = cwq[fc * 128 : (fc + 1) * 128,
                      2 * ccp * 128 : (2 * ccp + 2) * 128, kk]  # [f, 2*128]
            # target [p, c*128 + f] = blk[f, c*128 + p]
            wts[:, i * 256 : (i + 1) * 256] = (
                blk.reshape(128, 2, 128).transpose(2, 1, 0).reshape(128, 256)
            )
            i += 1

    # emb8 viewed [pair, c, p, batch, t]
    ev = emb8.reshape(NPAIR, 2, 128, B, S)
    in_maps = []
    for c in range(NCORES):
        v = ev[:, :, :, c * BL : (c + 1) * BL, :].reshape(NPAIR, 2, 128, 2, 8, S)
        # [pair, c2, p, h, b, t] -> [pair, p, h, c2, (b t)] half-streams
        hs = v.transpose(0, 2, 3, 1, 4, 5).reshape(NPAIR, 128, 2, 2, XH)
        # quad blocks: q = h*2+qq covers x [512qq, 512qq+544) of half h (zero
        # padded); blob is quad-major [q, pair, c, x] so one transfer carries
        # a full quad (both pairs) as contiguous 2176B per partition
        tmp = np.zeros((128, 2, 2, NPAIR, 2, XBLK), dtype=F8)  # [p,h,qq,pair,c,x]
        hsp = hs.transpose(1, 2, 0, 3, 4)  # [p, h, pair, c2, XH]
        tmp[:, :, 0, :, :, 0:XBLK] = hsp[..., 0:XBLK]
        tmp[:, :, 1, :, :, 0 : XH - 512] = hsp[..., 512:XH]
        in_maps.append({"emb": tmp.reshape(128, EMB_FREE), "wts": wts})

    nc = _get_compiled(need_min)
    trace = _maybe_enable_trace()
    res = run_bass_kernel_spmd(
        nc, in_maps, core_ids=list(range(NCORES)), trace=trace,
        tmpdir=os.environ.get("KERNEL_TRACE_DIR") or None,
    )
    _LAST_RESULTS = res

    # ---- host: combine stats -> BN -> pooled -> fc head (float64) ----
    FT = sum(FILTERS)  # 768
    inv = 1.0 / (SC_A * SC_W)
    cmax = np.empty((FT, B), dtype=np.float64)
    cmin = np.empty((FT, B), dtype=np.float64) if need_min else None
    sumsq = np.zeros(FT, dtype=np.float64)

    def bn_sumsq(bn):
        # bn_stats output: [cnt, mean, cnt*var] x (even, odd) half-streams
        return (bn[:, 2] + bn[:, 0] * bn[:, 1] ** 2
                + bn[:, 5] + bn[:, 3] * bn[:, 4] ** 2)

    for c in range(NCORES):
        stats = res.results[c]["stats"].astype(np.float64)  # [6, 128, nstat]
        for g, (bank, fc) in enumerate(GROUPS):
            ch = bank * 256 + fc * 128
            sl = slice(ch, ch + 128)
            pieces = (PIECES_LAST if g == len(GROUPS) - 1
                      else PIECES_G4 if g == len(GROUPS) - 2 else PIECES_FULL)
            bidx = 0
            for col, nb in pieces:
                bs = slice(c * BL + bidx, c * BL + bidx + nb)
                cmax[sl, bs] = stats[g, :, col : col + nb] * inv
                if g == len(GROUPS) - 1 and nb == 1:
                    sumsq[sl] += bn_sumsq(stats[g, :, col + 1 : col + 7]) * inv * inv
                else:
                    sumsq[sl] += stats[g, :, col + nb] * inv * inv
                if need_min:
                    cmin[sl, bs] = stats[g, :, NSTAT + bidx : NSTAT + bidx + nb] * inv
                bidx += nb

    # channel means via the factorized sum (exact: sum_t conv = w . window-sums)
    embT64 = embT.astype(np.float64)
    st_sum = embT64.sum(axis=1)                        # [512, S] summed over batch
    cum = np.concatenate(
        [np.zeros((CIN, 1)), np.cumsum(st_sum, axis=1)], axis=1
    )                                                  # [512, S+1]
    mean = np.empty(FT, dtype=np.float64)
    for bank in range(3):
        k, L = KS[bank], LS[bank]
        cw = conv_ws[bank].astype(np.float64)          # [256, 512, k]
        hs = np.stack([cum[:, kk + L] - cum[:, kk] for kk in range(k)], axis=1)
        mean[bank * 256 : (bank + 1) * 256] = (
            np.einsum("fck,ck->f", cw, hs) / (B * L)
        )

    counts = np.repeat([B * L for L in LS], FILTERS)
    var = sumsq / counts - mean * mean
    g_all = np.concatenate(bn_gs)
    b_all = np.concatenate(bn_bs)
    s = g_all / np.sqrt(var + EPS)
    shift = b_all - mean * s
    M = np.where(s[:, None] >= 0.0, cmax, cmin if need_min else cmax)  # [768, B]
    pooled = np.maximum(s[:, None] * M + shift[:, None], 0.0).T  # [B, 768]

    z = pooled @ np.asarray(fc1_w, dtype=np.float64) + np.asarray(
        fc1_b, dtype=np.float64
    )
    mu = z.mean(axis=0, keepdims=True)
    vz = np.square(z - mu).mean(axis=0, keepdims=True)
    z = (z - mu) / np.sqrt(vz + EPS) * np.asarray(
        bn1_g, dtype=np.float64
    ) + np.asarray(bn1_b, dtype=np.float64)
    z = np.maximum(z, 0.0)
    logits = z @ np.asarray(fc2_w, dtype=np.float64) + np.asarray(
        fc2_b, dtype=np.float64
    )
    logits -= logits.max(axis=1, keepdims=True)
    p = np.exp(logits)
    p /= p.sum(axis=1, keepdims=True)
    return p.astype(np.float32)


# revision 59
# speedup vs baseline: 1.0187x; 1.0156x over previous
"""Trainium2 Bass kernel for the char-CNN NLP model (data-parallel over 8 cores).

Pipeline:
  host:   emb = x @ emb_w (one-hot projection), laid out [cin, batch, seq],
          quantized to fp8e4 (scaled x64; TRN FP8_EXP4 == ml_dtypes.float8_e4m3)
  device: 3 parallel 1-D conv banks (k=2,3,4; 256 filters each) as fp8
          DoubleRow matmuls (two cin-chunks contracted per pass, fp32 PSUM);
          per (channel, batch) max over sequence; per channel sum of squares
          -> tiny stats tensor per core
  host:   batchnorm statistics from the factorized mean + device sumsq,
          monotone-affine BN+ReLU+maxpool reconstruction from max (min when
          some bn gamma < 0), fc1 -> bn -> relu -> fc2 -> softmax

BN(c+bias) is affine per channel, so max_t relu(bn(c)) = relu(s*M + t) with
M = max_t c if s>=0 else min_t c - exact, and the conv bias cancels inside BN.

Layout trick: each batch's sequence is stored at stride 128 (= S) with no
per-batch gap, so a conv tap at offset kk is one flat contiguous 512-wide
moving operand covering 4 batches; output columns t in [L, 128) accumulate
garbage that the evacuation slices away.

Perf notes (from trace analysis):
  - PE starts HAM-throttled at half clock and un-throttles only after ~5us
    of sustained high-duty matmul activity -> big junk warmup matmuls fill
    the initial DMA window (small ones don't count toward un-throttle).
  - HWDGE loads: ~0.1 B/ns per queue for >=1KB-per-partition elements, first
    packets ~1.4us after the dispatch instruction; loads are laddered across
    both queues in consumption order, with the k=4 bank first (most compute
    per embedding byte -> demand tracks supply with no stalls).
  - Stats leave as several small DMAs; the final ones are split by partition
    half across both queues since small-element DMAs are packet-rate-bound.
"""

import os
import numpy as np
import ml_dtypes

# ---------------- problem constants (hardcoded per contract) ----------------
B, S, W, V, E = 128, 128, 16, 128, 32
FILTERS = [256, 256, 256]
KS = [2, 3, 4]
NCLS = 10
EPS = 1e-5
NCORES = 8
BL = B // NCORES             # 16 batches per core
CIN = W * E                  # 512 conv input channels
NCC = CIN // 128             # 4 contraction chunks
NPAIR = NCC // 2             # 2 DoubleRow chunk pairs
LS = [S - k + 1 for k in KS]  # 127, 126, 125 valid conv positions
XH = 8 * 128                 # one batch-half (8 batches x 128) elems
XBLK = 544                   # quad block: 4 batches x 128 + 32 tap/pad slack
EMB_FREE = 8 * 2 * XBLK      # blob: quad-major (q, p, c, x) blocks = 8704
SC_A = 64.0                  # activation fp8 scale
SC_W = 64.0                  # weight fp8 scale
# group order: k=4 bank first (demand tracks DMA supply) and k=4 bank LAST:
# scalar-engine evacuation (square+accum ~0.97us/piece) exceeds a k=2 piece's
# 0.86us stream time, so k=2 groups interleave with k=3 groups (whose slack
# drains the backlog) and the stream ends on the k=4 bank whose evacuation
# drains in-line
GROUPS = [(2, 0), (0, 0), (1, 0), (0, 1), (1, 1), (2, 1)]
# per-group evacuation pieces: (stat block col, nb batches); last group ends
# with 2+1+1 batches so the tail only trails by single-batch pieces whose
# sum-of-squares comes from vector bn_stats (no scalar round-trip)
PIECES_FULL = [(0, 4), (5, 4), (10, 4), (15, 4)]
# second-to-last group: final piece at col 18 so its trailing stats share the
# [18:32] column window with the last group's and ride one fused DMA
PIECES_G4 = [(0, 4), (5, 4), (10, 4), (18, 4)]
PIECES_LAST = [(0, 4), (5, 4), (10, 4), (15, 2), (18, 1), (25, 1)]
NSTAT = 32                   # full groups use cols 0..19; last group: max 15
                             # sq 17 for (15,2); col 18 max + 19:25 bn_stats;
                             # col 25 max + 26:32 bn_stats
NWARM_SMALL = 2              # 128-col warmups
NWARM_BIG = 13               # 384-col warmups on the garbage region
# weight blob column offsets: groups laid out [g0, g1, g2, g5, g3, g4] so the
# weight transfers [g0], [g1 g2], [g5], [g3 g4] are contiguous
WOFF = [0, 2048, 3072, 6656, 7680, 4608]
F8 = ml_dtypes.float8_e4m3   # TRN FP8_EXP4: bias 7, max +-240

_CACHE = {}
_LAST_RESULTS = None


def _group_tiles(bank):
    return [(ccp, kk) for ccp in range(NPAIR) for kk in range(KS[bank])]


def _weight_tile_count():
    return sum(len(_group_tiles(bank)) for bank, _ in GROUPS)


def _build_bass(need_min):
    import concourse.tile as tile
    from concourse import bacc, mybir
    from contextlib import ExitStack

    nc = bacc.Bacc("TRN2", target_bir_lowering=False, debug=False, enable_asserts=False)

    ntiles = _weight_tile_count()  # 36 DoubleRow tiles of [128, 2, 128]
    nstat = NSTAT + (16 if need_min else 0)
    DR = mybir.MatmulPerfMode.DoubleRow
    emb_d = nc.dram_tensor(
        "emb", [128, EMB_FREE], mybir.dt.float8e4, kind="ExternalInput"
    ).ap()
    wts_d = nc.dram_tensor(
        "wts", [128, ntiles * 256], mybir.dt.float8e4, kind="ExternalInput"
    ).ap()
    stats_d = nc.dram_tensor(
        "stats", [len(GROUPS), 128, nstat], mybir.dt.float32, kind="ExternalOutput"
    ).ap()

    # ---- raw SBUF buffers + pre-TileContext loads: dma_start issued right
    # after engine init (before the TileContext entry barrier) starts the
    # HWDGE queues ~0.8us earlier than in-context loads. Completion is
    # tracked manually: each transfer incs its queue's semaphore by 16;
    # first-consumer matmuls carry wait_op(sem >= 16*k). ----
    warm_t = nc.alloc_sbuf_tensor("warm_raw", [128, 1028], mybir.dt.float8e4)
    emb_t = nc.alloc_sbuf_tensor("embr", [128, EMB_FREE], mybir.dt.float8e4)
    wt_t = nc.alloc_sbuf_tensor("wtr", [128, ntiles * 256], mybir.dt.float8e4)
    # one semaphore PER TRANSFER: a queue's 16 DMA engines start staggered
    # and interleave chunks of consecutive transfers, so a shared per-queue
    # counter hitting 16*k does NOT mean the k-th transfer finished. Few,
    # large transfers amortize the ~1.2us engine-stagger completion latency.
    SEMS = []

    def _load(eng, dst_t, c0, c1, src):
        s = nc.alloc_semaphore(f"ld{len(SEMS)}")
        SEMS.append(s)
        eng.dma_start(dst_t.ap()[:, c0:c1], src[:, c0:c1]).then_inc(s, 16)
        return (len(SEMS) - 1, 16)

    # laddered in consumption order across both queues; quad 0 is split into
    # its two pair-blocks (one per queue) so the stream starts on ~400KB, the
    # rest ride fused transfers to amortize the per-transfer stagger
    w_g0 = _load(nc.sync, wt_t, 0, 2048, wts_d)                 # g0 (k=4)
    e_p0q0 = _load(nc.scalar, emb_t, 0, 1088, emb_d)
    e_p1q0 = _load(nc.sync, emb_t, 1088, 2176, emb_d)
    e_q1 = _load(nc.scalar, emb_t, 2176, 4352, emb_d)
    e_q2 = _load(nc.sync, emb_t, 4352, 6528, emb_d)
    e_q3 = _load(nc.scalar, emb_t, 6528, 8704, emb_d)
    w_34 = _load(nc.sync, wt_t, 6656, 9216, wts_d)              # g3, g4
    w_12 = _load(nc.scalar, wt_t, 2048, 4608, wts_d)            # g1, g2
    w_5 = _load(nc.sync, wt_t, 4608, 6656, wts_d)               # g5
    EMB_W = [
        [e_p0q0, e_q1, e_q2, e_q3],
        [e_p1q0, e_q1, e_q2, e_q3],
    ]
    WT_W = [[w_g0], [w_12], [w_12], [w_34], [w_34], [w_5]]
    pending_waits = []

    with tile.TileContext(nc) as tc, ExitStack() as ctx:
        psum_pool = ctx.enter_context(tc.tile_pool(name="psum", bufs=7, space="PSUM"))
        stats_pool = ctx.enter_context(tc.tile_pool(name="stats", bufs=3))
        scr_pool = ctx.enter_context(tc.tile_pool(name="scr", bufs=4))

        # ---- PE warmup: junk DoubleRow matmuls while input DMAs stream, so
        # HAM un-throttles before/near the real stream start (PE boots at
        # half clock; only sustained full-width activity re-ramps it, and an
        # idle gap resets the credit). Operands are uninitialized raw-SBUF
        # garbage — fp8e4 has no inf, and NaN products land in a junk PSUM
        # tile nothing reads — so the first warmup issues the moment the PE
        # enters the kernel body. ----
        warm = warm_t.ap()
        wlhs = warm[:, :256].rearrange("p (c f) -> p c f", c=2)
        wrhs_s = warm[:, :256].rearrange("p (c x) -> p c x", c=2)     # x=128
        wrhs_b = warm[:, 260:1028].rearrange("p (c x) -> p c x", c=2)  # x=384
        wps = psum_pool.tile(
            [128, 384], mybir.dt.float32, tag="warm", bufs=1, name="wps"
        )
        for _ in range(NWARM_SMALL):
            nc.tensor.matmul(
                wps[:, :128], wlhs, wrhs_s, start=True, stop=True, perf_mode=DR
            )
        for _ in range(NWARM_BIG):
            nc.tensor.matmul(
                wps[:], wlhs, wrhs_b, start=True, stop=True, perf_mode=DR
            )

        def rhs_ap(ccp, q, kk, boff, nb):
            # moving operand [128, 2, nb*128]: dim1 steps between the two
            # chunks of the pair, free covers nb batches at stride 128
            off = (2 * q + ccp) * 2 * XBLK
            src = emb_t.ap()[:, off : off + 2 * XBLK].rearrange(
                "p (c x) -> p c x", c=2
            )
            x0 = boff * 128 + kk
            return src[:, :, x0 : x0 + nb * 128]

        # ---- conv banks: quad-major so each accumulation group stops early
        # and its evacuation overlaps the next quad's matmul stream. The last
        # group's final 2+1+1 batches run at the very end so only
        # single-batch evacuations + small split stats DMAs trail. ----
        # the last two groups share one tile so their trailing [18:32] stats
        # can leave as a single fused strided DMA pair
        st45 = stats_pool.tile(
            [128, 2 * nstat], mybir.dt.float32, tag="st45", bufs=1, name="st45"
        )
        st_sb = [
            stats_pool.tile([128, nstat], mybir.dt.float32, tag="st", name=f"st{g}")
            for g in range(len(GROUPS) - 2)
        ] + [st45[:, 0:nstat], st45[:, nstat : 2 * nstat]]

        def merged_waits(wlists):
            best = {}
            for sem_idx, val in wlists:
                best[sem_idx] = max(best.get(sem_idx, 0), val)
            return list(best.items())

        def emit_mms(g, bank, bidx, nb, pi):
            tiles = _group_tiles(bank)
            wt = wt_t.ap()
            q = bidx // 4
            boff = bidx - q * 4
            half = len(tiles) // 2
            w0 = merged_waits([EMB_W[0][q]] + WT_W[g][:1])
            w1 = merged_waits([EMB_W[1][q]] + WT_W[g][1:])
            ps = psum_pool.tile(
                [128, nb * 128], mybir.dt.float32, tag="ps", name=f"ps{g}_{pi}"
            )
            for i, (ccp, kk) in enumerate(tiles):
                lhs = wt[:, WOFF[g] + i * 256 : WOFF[g] + (i + 1) * 256].rearrange(
                    "p (c f) -> p c f", c=2
                )
                inst = nc.tensor.matmul(
                    ps[:], lhs, rhs_ap(ccp, q, kk, boff, nb),
                    start=(i == 0), stop=(i == len(tiles) - 1), perf_mode=DR,
                )
                # manual raw-load gating: pre-context DMAs are invisible to
                # the tile dependency tracker (and its deadlock-checking
                # simulator), so stash the waits and attach them after the
                # context is scheduled
                for sem_idx, val in (w0 if i == 0 else w1 if i == half else []):
                    pending_waits.append((inst, sem_idx, val))
            L = LS[bank]
            return ps[:].rearrange("p (b t) -> p b t", t=128)[:, :, :L]

        def evac_max_sq(g, col, nb, bidx, pv, L):
            st = st_sb[g]
            nc.vector.tensor_reduce(
                st[:, col : col + nb], pv, axis=mybir.AxisListType.X,
                op=mybir.AluOpType.max,
            )
            if need_min:
                nc.vector.tensor_reduce(
                    st[:, NSTAT + bidx : NSTAT + bidx + nb], pv,
                    axis=mybir.AxisListType.X, op=mybir.AluOpType.min,
                )
            scr = scr_pool.tile([128, 512], mybir.dt.float32, tag="scr")
            scr_v = scr[:, : nb * L].rearrange("p (b t) -> p b t", t=L)
            nc.scalar.activation(
                scr_v, pv,
                mybir.ActivationFunctionType.Square,
                accum_out=st[:, col + nb : col + nb + 1],
            )

        def emit_piece(g, bank, col, nb, bidx, pi):
            pv = emit_mms(g, bank, bidx, nb, pi)
            evac_max_sq(g, col, nb, bidx, pv, LS[bank])

        def emit_group_pieces(g, pieces, bidx0):
            bank, _ = GROUPS[g]
            bidx = bidx0
            for pi, (col, nb) in enumerate(pieces):
                emit_piece(g, bank, col, nb, bidx, f"{pi}_{bidx}")
                bidx += nb

        def dma_stats(eng, g, c0, c1):
            eng.dma_start(stats_d[g][:, c0:c1], st_sb[g][:, c0:c1])

        def dma_stats_split(g, c0, c1):
            # partition-halved across two queues: small-elem DMAs are
            # packet-rate-bound, so two halves finish ~2x sooner. Stats ride
            # sync/gpsimd only — a dispatch costs ~0.6us of ENGINE time, and
            # the scalar engine is the evacuation bottleneck
            nc.sync.dma_start(stats_d[g][0:64, c0:c1], st_sb[g][0:64, c0:c1])
            nc.gpsimd.dma_start(stats_d[g][64:128, c0:c1], st_sb[g][64:128, c0:c1])

        glast = len(GROUPS) - 1
        for g in range(glast - 1):
            emit_group_pieces(g, PIECES_FULL, 0)
            dma_stats(nc.sync if g % 2 == 0 else nc.gpsimd, g, 0, 20)
            if need_min:
                dma_stats(nc.sync if g % 2 == 0 else nc.gpsimd, g, NSTAT, NSTAT + 16)
        # last group's first 14 batches run before the second-to-last group,
        # so their evacuations overlap that group's stream
        bank5 = GROUPS[glast][0]
        L5 = LS[bank5]
        for pi, (col, nb) in enumerate(PIECES_LAST[:4]):
            emit_piece(glast, bank5, col, nb, sum(p[1] for p in PIECES_LAST[:pi]),
                       f"l{pi}")
        dma_stats(nc.gpsimd, glast, 0, 18)
        if need_min:
            dma_stats(nc.gpsimd, glast, NSTAT, NSTAT + 14)
        emit_group_pieces(glast - 1, PIECES_G4, 0)
        dma_stats_split(glast - 1, 0, 15)
        if need_min:
            dma_stats(nc.sync, glast - 1, NSTAT, NSTAT + 16)

        # trailing single-batch pieces: max on gpsimd/vector, sum of squares
        # via vector bn_stats -- no scalar-engine round trip at the tail
        st5 = st_sb[glast]
        pv14 = emit_mms(glast, bank5, 14, 1, "l4")
        nc.vector.tensor_reduce(
            st5[:, 18:19], pv14, axis=mybir.AxisListType.X, op=mybir.AluOpType.max
        )
        nc.vector.bn_stats(st5[:, 19:25], pv14[:, 0, :])
        if need_min:
            nc.vector.tensor_reduce(
                st5[:, NSTAT + 14 : NSTAT + 15], pv14,
                axis=mybir.AxisListType.X, op=mybir.AluOpType.min,
            )
        pv15 = emit_mms(glast, bank5, 15, 1, "l5")
        nc.vector.tensor_reduce(
            st5[:, 25:26], pv15, axis=mybir.AxisListType.X, op=mybir.AluOpType.max
        )
        nc.vector.bn_stats(st5[:, 26:32], pv15[:, 0, :])
        if need_min:
            nc.vector.tensor_reduce(
                st5[:, NSTAT + 15 : NSTAT + 16], pv15,
                axis=mybir.AxisListType.X, op=mybir.AluOpType.min,
            )
        # final fused stats: one strided DMA pair carries the [18:32] window
        # of BOTH trailing groups (they share the st45 tile)
        src45 = st45[:].rearrange("p (g c) -> p g c", g=2)[:, :, 18:32]
        dst45 = stats_d[glast - 1 : glast + 1, :, 18:32].rearrange(
            "g p c -> p g c"
        )
        nc.sync.dma_start(dst45[0:64], src45[0:64])
        nc.scalar.dma_start(dst45[64:128], src45[64:128])
        if need_min:
            dma_stats(nc.gpsimd, glast, NSTAT + 14, nstat)

    # attach the raw-load gating waits now that tile scheduling is done;
    # check=False since Bacc's generate_event_semaphores pass legalizes
    # instructions carrying more than one wait
    for inst, sem_idx, val in pending_waits:
        inst.wait_op(SEMS[sem_idx], val, "sem-ge", check=False)

    nc.compile()
    return nc


def _get_compiled(need_min):
    key = ("nc", need_min)
    if key not in _CACHE:
        _CACHE[key] = _build_bass(need_min)
    return _CACHE[key]


def _maybe_enable_trace():
    if os.environ.get("KERNEL_TRACE") != "1":
        return False
    try:
        import sys, types

        if "antenv.axon_hooks" not in sys.modules:
            mod = types.ModuleType("antenv.axon_hooks")
            _h = {"hook": None}
            mod.set_axon_ntff_profile_hook = lambda h: _h.__setitem__("hook", h)
            mod.get_axon_ntff_profile_hook = lambda: _h["hook"]
            sys.modules["antenv.axon_hooks"] = mod
            import antenv

            antenv.axon_hooks = mod
            from trn_agent_boot.trn_boot import _ntff_profile_via_ctypes

            mod.set_axon_ntff_profile_hook(
                _ntff_profile_via_ctypes("/opt/axon/libaxon_pjrt.so")
            )
        import concourse.bass_utils as bu

        bu.upload_artifacts = lambda tmpdir: tmpdir
        return True
    except Exception:
        return False


def _q8(a, sc):
    return np.clip(np.asarray(a, dtype=np.float32) * sc, -240.0, 240.0).astype(F8)


def kernel(
    x, emb_w,
    conv_w0, conv_b0, bn_g0, bn_b0,
    conv_w1, conv_b1, bn_g1, bn_b1,
    conv_w2, conv_b2, bn_g2, bn_b2,
    fc1_w, fc1_b, bn1_g, bn1_b, fc2_w, fc2_b,
):
    global _LAST_RESULTS
    from concourse.bass_utils import run_bass_kernel_spmd

    x = np.asarray(x, dtype=np.float32)
    emb_w = np.asarray(emb_w, dtype=np.float32)
    conv_ws = [np.asarray(w, dtype=np.float32) for w in (conv_w0, conv_w1, conv_w2)]
    bn_gs = [np.asarray(v, dtype=np.float64) for v in (bn_g0, bn_g1, bn_g2)]
    bn_bs = [np.asarray(v, dtype=np.float64) for v in (bn_b0, bn_b1, bn_b2)]
    need_min = bool((np.concatenate(bn_gs) < 0.0).any())

    # ---- host: embedding (x is one-hot in practice; dense matmul is exact) ----
    e = x.reshape(-1, V) @ emb_w                       # [B*S*W, E]
    e = e.reshape(B, S, CIN)                           # [B, S, 512]
    embT = np.ascontiguousarray(e.transpose(2, 0, 1))  # [512, B, S]
    emb8 = _q8(embT, SC_A)                             # [512, B, 128]

    # ---- pack device inputs (weights at blob offsets WOFF) ----
    ntiles = _weight_tile_count()
    wts = np.empty((128, ntiles * 256), dtype=F8)
    for g, (bank, fc) in enumerate(GROUPS):
        cwq = _q8(conv_ws[bank], SC_W)                 # [256, 512, k]
        i = WOFF[g] // 256
        for ccp, kk in _group_tiles(bank):
            blk = cwq[fc * 128 : (fc + 1) * 128,
                      2 * ccp * 128 : (2 * ccp + 2) * 128, kk]  # [f, 2*128]
            # target [p, c*128 + f] = blk[f, c*128 + p]
            wts[:, i * 256 : (i + 1) * 256] = (
                blk.reshape(128, 2, 128).transpose(2, 1, 0).reshape(128, 256)
            )
            i += 1

    # emb8 viewed [pair, c, p, batch, t]
    ev = emb8.reshape(NPAIR, 2, 128, B, S)
    in_maps = []
    for c in range(NCORES):
        v = ev[:, :, :, c * BL : (c + 1) * BL, :].reshape(NPAIR, 2, 128, 2, 8, S)
        # [pair, c2, p, h, b, t] -> [pair, p, h, c2, (b t)] half-streams
        hs = v.transpose(0, 2, 3, 1, 4, 5).reshape(NPAIR, 128, 2, 2, XH)
        # quad blocks: q = h*2+qq covers x [512qq, 512qq+544) of half h (zero
        # padded); blob is quad-major [q, pair, c, x] so one transfer carries
        # a full quad (both pairs) as contiguous 2176B per partition
        tmp = np.zeros((128, 2, 2, NPAIR, 2, XBLK), dtype=F8)  # [p,h,qq,pair,c,x]
        hsp = hs.transpose(1, 2, 0, 3, 4)  # [p, h, pair, c2, XH]
        tmp[:, :, 0, :, :, 0:XBLK] = hsp[..., 0:XBLK]
        tmp[:, :, 1, :, :, 0 : XH - 512] = hsp[..., 512:XH]
        in_maps.append({"emb": tmp.reshape(128, EMB_FREE), "wts": wts})

    nc = _get_compiled(need_min)
    trace = _maybe_enable_trace()
    res = run_bass_kernel_spmd(
        nc, in_maps, core_ids=list(range(NCORES)), trace=trace,
        tmpdir=os.environ.get("KERNEL_TRACE_DIR") or None,
    )
    _LAST_RESULTS = res

    # ---- host: combine stats -> BN -> pooled -> fc head (float64) ----
    FT = sum(FILTERS)  # 768
    inv = 1.0 / (SC_A * SC_W)
    cmax = np.empty((FT, B), dtype=np.float64)
    cmin = np.empty((FT, B), dtype=np.float64) if need_min else None
    sumsq = np.zeros(FT, dtype=np.float64)

    def bn_sumsq(bn):
        # bn_stats output: [cnt, mean, cnt*var] x (even, odd) half-streams
        return (bn[:, 2] + bn[:, 0] * bn[:, 1] ** 2
                + bn[:, 5] + bn[:, 3] * bn[:, 4] ** 2)

    for c in range(NCORES):
        stats = res.results[c]["stats"].astype(np.float64)  # [6, 128, nstat]
        for g, (bank, fc) in enumerate(GROUPS):
            ch = bank * 256 + fc * 128
            sl = slice(ch, ch + 128)
            pieces = (PIECES_LAST if g == len(GROUPS) - 1
                      else PIECES_G4 if g == len(GROUPS) - 2 else PIECES_FULL)
            bidx = 0
            for col, nb in pieces:
                bs = slice(c * BL + bidx, c * BL + bidx + nb)
                cmax[sl, bs] = stats[g, :, col : col + nb] * inv
                if g == len(GROUPS) - 1 and nb == 1:
                    sumsq[sl] += bn_sumsq(stats[g, :, col + 1 : col + 7]) * inv * inv
                else:
                    sumsq[sl] += stats[g, :, col + nb] * inv * inv
                if need_min:
                    cmin[sl, bs] = stats[g, :, NSTAT + bidx : NSTAT + bidx + nb] * inv
                bidx += nb

    # channel means via the factorized sum (exact: sum_t conv = w . window-sums)
    embT64 = embT.astype(np.float64)
    st_sum = embT64.sum(axis=1)                        # [512, S] summed over batch
    cum = np.concatenate(
        [np.zeros((CIN, 1)), np.cumsum(st_sum, axis=1)], axis=1
    )                                                  # [512, S+1]
    mean = np.empty(FT, dtype=np.float64)
    for bank in range(3):
        k, L = KS[bank], LS[bank]
        cw = conv_ws[bank].astype(np.float64)          # [256, 512, k]
        hs = np.stack([cum[:, kk + L] - cum[:, kk] for kk in range(k)], axis=1)
        mean[bank * 256 : (bank + 1) * 256] = (
            np.einsum("fck,ck->f", cw, hs) / (B * L)
        )

    counts = np.repeat([B * L for L in LS], FILTERS)
    var = sumsq / counts - mean * mean
    g_all = np.concatenate(bn_gs)
    b_all = np.concatenate(bn_bs)
    s = g_all / np.sqrt(var + EPS)
    shift = b_all - mean * s
    M = np.where(s[:, None] >= 0.0, cmax, cmin if need_min else cmax)  # [768, B]
    pooled = np.maximum(s[:, None] * M + shift[:, None], 0.0).T  # [B, 768]

    z = pooled @ np.asarray(fc1_w, dtype=np.float64) + np.asarray(
        fc1_b, dtype=np.float64
    )
    mu = z.mean(axis=0, keepdims=True)
    vz = np.square(z - mu).mean(axis=0, keepdims=True)
    z = (z - mu) / np.sqrt(vz + EPS) * np.asarray(
        bn1_g, dtype=np.float64
    ) + np.asarray(bn1_b, dtype=np.float64)
    z = np.maximum(z, 0.0)
    logits = z @ np.asarray(fc2_w, dtype=np.float64) + np.asarray(
        fc2_b, dtype=np.float64
    )
    logits -= logits.max(axis=1, keepdims=True)
    p = np.exp(logits)
    p /= p.sum(axis=1, keepdims=True)
    return p.astype(np.float32)
